# revision 29
# baseline (speedup 1.0000x reference)
"""Trainium2 Bass kernel for nn_DigitConvolutionalModel (3x3 valid conv + 3-layer MLP).

Strategy
--------
The 3x3 "valid" cross-correlation is linear in x, so it is folded on the host
into the first MLP weight:  conv(x).reshape(B, 676) @ w1  ==  x @ weff  with
weff[784, 256] built from conv_w and w1.  The device then runs a pure 3-layer
MLP:

    out = relu(relu(x @ weff + b1) @ w2 + b2) @ w3 + b3

Data-parallel over the batch across 8 NeuronCores (8192 rows per core).
On-chip dataflow is feature-major ([features, batch] tiles) so the contraction
dim of every matmul lands on SBUF partitions with zero on-chip transposes; the
host pre-tiles each x shard into the exact SBUF layout so every device DMA is
a fully contiguous HBM read, and transposes the [10, batch] result back.

The matmul path runs in float16 (fp32 PSUM accumulation): x and the weights
are rounded to fp16 on the host.  Measured end-to-end relative error vs the
fp32 reference is ~5e-4 (fp16's 10 mantissa bits; activations are O(10) so
there is no range risk).  fp16 halves DMA bytes and enables full-rate PE
matmuls with fast weight loads.

Schedule: a software pipeline L1(n) | L2(n-1) | L3(n-2) with explicit PE
issue-order edges so the in-order PE queue never waits on ACT/DVE epilogues.
The k-loop is outermost within L1 so the first matmul only needs the k=0
slices of weff and x[0]; those are DMA'd as separate small transfers so the
PE starts real work as early as possible.  A short burst of small dummy
matmuls at kernel start keeps the PE busy from the end of the framework
prologue until the first data lands, lifting the HAM clock gate to full rate
(4096-cycle activity window) before the bulk of the stream.
"""

import numpy as np

import concourse.bass as bass
import concourse.mybir as mybir
import concourse.tile as tile
from bass_rust import add_dep_helper
from concourse import bacc
from concourse.bass_utils import run_bass_kernel_spmd

N_CORES = 8
B = 65536
BS = B // N_CORES          # 8192 batch rows per core
KIN = 784                  # input features (28*28)
KC, KCH = 7, 112           # layer-1 contraction chunks: 7 x 112 = 784
H1, H2, NOUT = 256, 128, 10
NB = 512                   # batch tile (matmul free dim = one PSUM bank of fp32)
NITER = BS // NB           # 16
NWARM = 66                 # dummy warmup matmuls (N=128 each); their span must
                           # cover >= 2 HAM activity windows (2 x 3.41us) so a
                           # fully-covered aligned window exists for any phase
                           # of the free-running HAM counter

F32 = mybir.dt.float32
F16 = mybir.dt.float16
RELU = mybir.ActivationFunctionType.Relu


def build_program():
    nc = bacc.Bacc(
        "TRN2", target_bir_lowering=False, debug=False, num_devices=N_CORES
    )
    # all tensors arrive pre-tiled from the host in the exact SBUF layout so
    # every DMA reads DRAM fully contiguously (max HBM burst efficiency)
    xt_d = nc.dram_tensor("xt", [NITER, KCH, KC, NB], F16,
                          kind="ExternalInput").ap()
    weff_d = nc.dram_tensor("weff", [KCH, KC, H1], F16,
                            kind="ExternalInput").ap()
    w2_d = nc.dram_tensor("w2", [128, 2, H2], F16, kind="ExternalInput").ap()
    w3_d = nc.dram_tensor("w3", [H2, NOUT], F16, kind="ExternalInput").ap()
    bb_d = nc.dram_tensor("bb", [128, 3], F32, kind="ExternalInput").ap()
    out_d = nc.dram_tensor("out", [NITER, NOUT, NB], F32,
                           kind="ExternalOutput").ap()

    with tile.TileContext(nc) as tc:
        with (
            tc.tile_pool(name="w", bufs=1) as wp,
            tc.tile_pool(name="x", bufs=6) as xp,
            tc.tile_pool(name="h", bufs=4) as hp,
            tc.tile_pool(name="o", bufs=4) as op,
            tc.tile_pool(name="ps", bufs=2, space=bass.MemorySpace.PSUM) as pp,
        ):
            # HAM warmup: dummy matmuls keep the PE busy from the end of the
            # framework prologue until the first x/weff slices land, so the
            # HAM activity window lifts the clock gate to 8/8 early.  N=128
            # keeps each one short so queued warmups never delay real work
            # by much.  Operand data is never consumed; the one-column memset
            # is the cheapest possible writer so Tile allocates the tile.
            warm = wp.tile([KCH, 128], F16, tag="warm")
            nc.gpsimd.memset(warm[:, 0:1], 0.0)
            pw = pp.tile([128, NB], F32, tag="p1_0")
            last_mm = None  # previous PE instruction, for ordering edges
            warm_mms = []
            for _ in range(NWARM):
                mm = nc.tensor.matmul(pw[:, 0:128], warm[:, 0:128], warm[:],
                                      start=True, stop=True)
                if last_mm is not None:
                    add_dep_helper(mm.ins, last_mm.ins, sync=False,
                                   reason="PE issue order")
                last_mm = mm
                warm_mms.append(mm)

            # weff rides the scalar queue, which is otherwise idle until the
            # first ACT (~16us): it lands by ~8us without stealing any DMA
            # bandwidth from the x-tile supply.  The sync queue is dedicated
            # to x tiles: keeping ALL bulk x DMAs on one in-order queue is
            # deliberate — the DMA engines fair-share packet service across
            # queues, so concurrent bulk prefetches on a second queue starve
            # the critical next tile (measured: a 228KB slice took 10us while
            # 3 prefetched tiles round-robined on the engine pool).  Serial
            # per-queue order self-throttles the prefetch.
            # single contiguous transfers (7KB/3.5KB per partition) — split or
            # tiny DMAs generate sub-1KB packets that clog the engine pool's
            # packet slots during the critical early window
            # DMA engines round-robin service per PACKET across everything
            # outstanding, so even tiny DMAs (128 packets of 12B) steal as
            # many service slots as a bulk tile.  Give xt0 exclusive use of
            # the engines first: weff starts mid-warmup, and the small
            # weight/bias transfers (needed ~18us) start after the first
            # real matmul.
            weff_t = wp.tile([KCH, KC, H1], F16, tag="weff")
            dma = nc.scalar.dma_start(weff_t[:], weff_d[:])
            add_dep_helper(dma.ins, warm_mms[28].ins, sync=True,
                           reason="pace weff behind xt0")
            w2_t = wp.tile([128, 2, H2], F16, tag="w2")
            w2_dma = nc.gpsimd.dma_start(w2_t[:], w2_d[:])
            w3_t = wp.tile([H2, NOUT], F16, tag="w3")
            w3_dma = nc.gpsimd.dma_start(w3_t[:], w3_d[:])
            bb_t = wp.tile([128, 3], F32, tag="bb")  # b1 (2 cols) | b2 (1 col)
            bb_dma = nc.gpsimd.dma_start(bb_t[:], bb_d[:])
            small_dmas = [w2_dma, w3_dma, bb_dma]

            # software pipeline: L1(t) | L2(t-1) | L3(t-2) so the in-order PE
            # queue never waits on the ACT/DVE epilogues of the same tile.
            # The final iteration is split into two half-width tiles so the
            # serial ACT->L2->TS->L3->COPY->DMA drain at the end is short.
            tiles = [(n, 0, NB) for n in range(NITER - 1)]
            tiles += [(NITER - 1, 0, NB // 2), (NITER - 1, NB // 2, NB // 2)]
            NT = len(tiles)
            h1_hist = {}
            h2_hist = {}
            # The DMA engines round-robin packet service across ALL
            # outstanding descriptors (even within one queue ring), so
            # unpaced prefetch delays the critical next tile: with 6 tiles
            # in flight the first tile's completion slipped past the warmup
            # drain.  Pace tile t's DMA issue on the first matmul of tile
            # t-2 (warmup matmul #30 for t=1) so at most ~1 bulk transfer
            # is in flight and each tile gets near-exclusive bandwidth.
            pace_mm = {1: warm_mms[34], 2: warm_mms[58]}
            for t in range(NT + 2):
                if t < NT:
                    n, c0, w = tiles[t]
                    xt = xp.tile([KCH, KC, NB], F16, tag="x")
                    if w == NB:
                        dma = nc.sync.dma_start(xt[:], xt_d[n])
                    else:
                        dma = nc.sync.dma_start(xt[:, :, 0:w],
                                                xt_d[n][:, :, c0:c0 + w])
                    if t in pace_mm:
                        add_dep_helper(dma.ins, pace_mm[t].ins, sync=True,
                                       reason="pace x prefetch")
                    # k-outer so the first matmuls depend only on k=0 data
                    p1a = pp.tile([128, NB], F32, tag="p1_0")
                    p1b = pp.tile([128, NB], F32, tag="p1_1")
                    p1s = [p1a, p1b]
                    for k in range(KC):
                        for m in range(2):
                            mm = nc.tensor.matmul(
                                p1s[m][:, 0:w],
                                weff_t[:, k, m * 128:(m + 1) * 128],
                                xt[:, k, 0:w],
                                start=(k == 0),
                                stop=(k == KC - 1),
                            )
                            if last_mm is not None:
                                add_dep_helper(mm.ins, last_mm.ins, sync=False,
                                               reason="PE issue order")
                            last_mm = mm
                            if k == 0 and m == 0 and t == 0:
                                for sd in small_dmas:
                                    add_dep_helper(sd.ins, mm.ins, sync=True,
                                                   reason="pace small weights")
                            if k == 3 and m == 0:
                                pace_mm[t + 3] = mm
                    h1s = []
                    for m in range(2):
                        h1 = hp.tile([128, NB], F16, tag=f"h1_{m}")
                        nc.scalar.activation(
                            h1[:, 0:w], p1s[m][:, 0:w], RELU,
                            bias=bb_t[:, m:m + 1]
                        )
                        h1s.append(h1)
                    h1_hist[t] = h1s
                if 0 <= t - 1 < NT:
                    n, c0, w = tiles[t - 1]
                    h1s = h1_hist.pop(t - 1)
                    p2 = pp.tile([128, NB], F32, tag="p2")
                    for k in range(2):
                        mm = nc.tensor.matmul(
                            p2[:, 0:w],
                            w2_t[:, k, :],
                            h1s[k][:, 0:w],
                            start=(k == 0),
                            stop=(k == 1),
                        )
                        add_dep_helper(mm.ins, last_mm.ins, sync=False,
                                       reason="PE issue order")
                        last_mm = mm
                    h2 = hp.tile([128, NB], F16, tag="h2")
                    nc.vector.tensor_scalar(
                        h2[:, 0:w], p2[:, 0:w], bb_t[:, 2:3], 0.0,
                        mybir.AluOpType.add, mybir.AluOpType.max,
                    )
                    h2_hist[t - 1] = h2
                if 0 <= t - 2 < NT:
                    n, c0, w = tiles[t - 2]
                    h2 = h2_hist.pop(t - 2)
                    p3 = pp.tile([NOUT, NB], F32, tag="p3")
                    mm = nc.tensor.matmul(
                        p3[:, 0:w], w3_t[:], h2[:, 0:w], start=True, stop=True,
                    )
                    add_dep_helper(mm.ins, last_mm.ins, sync=False,
                                   reason="PE issue order")
                    last_mm = mm
                    ot = op.tile([NOUT, NB], F32, tag="ot")
                    nc.vector.tensor_copy(ot[:, 0:w], p3[:, 0:w])
                    # out DMAs ride the scalar queue: hardware DGE completion
                    # posts faster than the gpsimd software ring, and the
                    # sync queue stays dedicated to x-tile supply
                    nc.scalar.dma_start(out_d[n][:, c0:c0 + w], ot[:, 0:w])

    nc.compile()
    return nc


_NC = None


def _get_program():
    global _NC
    if _NC is None:
        _NC = build_program()
    return _NC


def make_in_maps(x, conv_w, w1, b1, w2, b2, w3, b3):
    """Host-side prep: fold conv into w1, pre-tile everything into the exact
    on-chip layout so device DMAs are fully contiguous."""
    conv_w = np.asarray(conv_w, np.float64)
    w1r = np.asarray(w1, np.float64).reshape(26, 26, H1)
    weff = np.zeros((28, 28, H1), np.float64)
    for u in range(3):
        for v in range(3):
            weff[u:u + 26, v:v + 26, :] += conv_w[u, v] * w1r
    weff = weff.reshape(KIN, H1).astype(np.float16)
    # [784, 256] -> [112, 7, 256]
    weff_d = np.ascontiguousarray(weff.reshape(KC, KCH, H1).transpose(1, 0, 2))
    # [256, 128] -> [128, 2, 128]
    w2_d = np.ascontiguousarray(
        np.asarray(w2, np.float16).reshape(2, 128, H2).transpose(1, 0, 2))

    bbd = np.ascontiguousarray(np.concatenate([
        np.asarray(b1, np.float32).reshape(2, 128).T,
        np.asarray(b2, np.float32).reshape(128, 1)], axis=1))
    w3c = np.ascontiguousarray(np.asarray(w3, np.float16))

    x = np.asarray(x, np.float16)
    in_maps = []
    for c in range(N_CORES):
        # [8192, 784] -> feature-major tiles [NITER, 112, 7, 512]
        xs = x[c * BS:(c + 1) * BS].T  # [784, 8192] view
        xs = np.ascontiguousarray(
            xs.reshape(KC, KCH, NITER, NB).transpose(2, 1, 0, 3))
        in_maps.append({
            "xt": xs, "weff": weff_d, "w2": w2_d, "w3": w3c,
            "bb": bbd,
        })
    return in_maps


def run(x, conv_w, w1, b1, w2, b2, w3, b3, trace=False):
    nc = _get_program()
    in_maps = make_in_maps(x, conv_w, w1, b1, w2, b2, w3, b3)
    br = run_bass_kernel_spmd(nc, in_maps, core_ids=list(range(N_CORES)),
                              trace=trace)
    out = np.empty((B, NOUT), np.float32)
    for c in range(N_CORES):
        # [NITER, 10, 512] -> [8192, 10]
        r = br.results[c]["out"]
        out[c * BS:(c + 1) * BS] = r.transpose(0, 2, 1).reshape(BS, NOUT)
    out += np.asarray(b3, np.float32)[None, :]
    return out, br


def kernel(x, conv_w, w1, b1, w2, b2, w3, b3):
    out, _ = run(x, conv_w, w1, b1, w2, b2, w3, b3)
    return out
